# revision 1
# baseline (speedup 1.0000x reference)
"""GraphSAGE 2-layer forward on 8 Trainium2 NeuronCores (Bass raw-block SPMD).

Strategy (per core c of 8):
- Layer 0 dsts [1375c, 1375(c+1)) in windows of 128. Edges sorted by dst,
  padded to 128-multiples per window. For each 128-edge tile: indirect-DMA
  gather of fp16 src rows [128, 602], DVE builds a value-onehot
  OH[p, d] = (iota == dstslot[p]) * (1/cnt[dst[p]]), PE accumulates
  aggT[featchunk, dst] += G_chunk.T @ OH into PSUM (5 chunks of 602).
  Self rows go through the same pipeline as one pseudo-tile per window with
  identity mapping and val=1 (plus a ones row for the bias).
- h[dst, 256] = relu(selfT.T @ [Wself0;b0] + aggT.T @ Wneigh0) per window,
  stored fp16 to a local DRAM slice; AllGather -> full h [11000, 256].
- Layer 1 = same machinery, one window of 125 dsts per core, K=256.
Output: per-core [125, 41] fp32 slices concatenated on host.
All per-core variation (indices, counts, values) is input data, so one SPMD
program serves all 8 cores.
"""

import numpy as np

P = 128
NCORES = 8

# full-size problem dims (hardcoded per spec)
N_SRC0, N_DST0, N_E0 = 286000, 11000, 275000
N_DST1, N_E1 = 1000, 10000
F_IN, N_HID, N_CLS = 602, 256, 41


def _chunks(k):
    """K-dim chunk sizes of size <=128 covering k."""
    out = []
    while k > 0:
        out.append(min(P, k))
        k -= P
    return out


def _pack_cols(arrs, ncols, dtype, fill=0):
    """Pack list of [128] columns into [128, ncols] array."""
    out = np.full((P, ncols), fill, dtype=dtype)
    for i, a in enumerate(arrs):
        out[: len(a), i] = a
    return out


def _prep_side(src, dst, n_dst_total, dst_per_core, self_rows_of, table_rows):
    """Sort/pad edges per core; emit per-core packed index/val arrays and the
    shared tile schedule. Returns (schedule, per_core_data).

    schedule: list of dicts(window, kind) shared by all cores.
    per_core: dict core -> (srcidx_cols, dstslot_cols, val_cols) one col/tile.
    """
    nwin = (dst_per_core + P - 1) // P
    cnt = np.bincount(dst, minlength=n_dst_total).astype(np.float64)
    cntinv = (1.0 / np.maximum(cnt, 1.0)).astype(np.float32)

    order = np.argsort(dst, kind="stable")
    src_s, dst_s = src[order], dst[order]
    core_of = dst_s // dst_per_core
    # per (core, window) edge lists
    percw = {}
    for c in range(NCORES):
        m = core_of == c
        sc, dc = src_s[m], dst_s[m]
        local = dc - c * dst_per_core
        w = local // P
        for wi in range(nwin):
            mm = w == wi
            percw[(c, wi)] = (sc[mm], (local[mm] - wi * P).astype(np.int64), dc[mm])

    # tiles per window = max over cores (same program everywhere), min 1
    tiles_w = [
        max(
            1,
            max((len(percw[(c, wi)][0]) + P - 1) // P for c in range(NCORES)),
        )
        for wi in range(nwin)
    ]

    schedule = []
    for wi in range(nwin):
        for _ in range(tiles_w[wi]):
            schedule.append({"w": wi, "kind": "agg"})
        schedule.append({"w": wi, "kind": "self"})

    per_core = {}
    for c in range(NCORES):
        s_cols, d_cols, v_cols = [], [], []
        for wi in range(nwin):
            es, eslot, edst = percw[(c, wi)]
            npad = tiles_w[wi] * P - len(es)
            s = np.concatenate([es, np.zeros(npad, np.int64)])
            dsl = np.concatenate([eslot, np.full(npad, -1, np.int64)])
            v = np.concatenate([cntinv[edst], np.zeros(npad, np.float32)])
            for t in range(tiles_w[wi]):
                sl = slice(t * P, (t + 1) * P)
                s_cols.append(s[sl])
                d_cols.append(dsl[sl])
                v_cols.append(v[sl])
            # self pseudo-tile: identity dst mapping, val=1
            ndst_w = min(P, dst_per_core - wi * P)
            selfrows = self_rows_of(c, wi, ndst_w)
            srow = np.zeros(P, np.int64)
            srow[:ndst_w] = selfrows
            drow = np.full(P, -1, np.int64)
            drow[:ndst_w] = np.arange(ndst_w)
            vrow = np.zeros(P, np.float32)
            vrow[:ndst_w] = 1.0
            s_cols.append(srow)
            d_cols.append(drow)
            v_cols.append(vrow)
        per_core[c] = (s_cols, d_cols, v_cols)
    return schedule, per_core, nwin


def _preprocess(x, Wself0, Wneigh0, b0, Wself1, Wneigh1, b1,
                e0_src, e0_dst, e1_src, e1_dst,
                n_src0, n_dst0, n_dst1, f_in, n_hid, n_cls):
    dpc0 = n_dst0 // NCORES
    dpc1 = n_dst1 // NCORES

    e0_src = np.asarray(e0_src).astype(np.int64)
    e0_dst = np.asarray(e0_dst).astype(np.int64)
    e1_src = np.asarray(e1_src).astype(np.int64)
    e1_dst = np.asarray(e1_dst).astype(np.int64)

    x16 = np.ascontiguousarray(np.asarray(x, dtype=np.float32).astype(np.float16))

    sched0, pc0, nwin0 = _prep_side(
        e0_src, e0_dst, n_dst0, dpc0,
        self_rows_of=lambda c, wi, n: c * dpc0 + wi * P + np.arange(n),
        table_rows=n_src0,
    )
    sched1, pc1, nwin1 = _prep_side(
        e1_src, e1_dst, n_dst1, dpc1,
        self_rows_of=lambda c, wi, n: c * dpc1 + wi * P + np.arange(n),
        table_rows=n_dst0,
    )
    assert nwin1 == 1

    ntiles0 = len(sched0)
    ntiles1 = len(sched1)
    ntiles = ntiles0 + ntiles1

    # weights: [Wself0; b0] -> [f_in+1, n_hid]; Wneigh0 [f_in, n_hid]
    W0s = np.concatenate([np.asarray(Wself0, np.float32),
                          np.asarray(b0, np.float32)[None, :]], 0).astype(np.float16)
    W0n = np.asarray(Wneigh0, np.float32).astype(np.float16)
    W1s = np.concatenate([np.asarray(Wself1, np.float32),
                          np.asarray(b1, np.float32)[None, :]], 0).astype(np.float16)
    W1n = np.asarray(Wneigh1, np.float32).astype(np.float16)

    in_maps = []
    for c in range(NCORES):
        s0, d0, v0 = pc0[c]
        s1, d1, v1 = pc1[c]
        srcidx = _pack_cols(s0 + s1, ntiles, np.int32)
        dstv = _pack_cols(d0 + d1, ntiles, np.float32)
        valv = _pack_cols(v0 + v1, ntiles, np.float32)
        in_maps.append({
            "x16": x16,
            "srcidx": srcidx,
            "dstv": dstv,
            "valv": valv,
            "W0s": W0s,
            "W0n": W0n,
            "W1s": W1s,
            "W1n": W1n,
        })

    params = dict(
        n_src0=n_src0, n_dst0=n_dst0, n_dst1=n_dst1,
        f_in=f_in, n_hid=n_hid, n_cls=n_cls,
        dpc0=dpc0, dpc1=dpc1, nwin0=nwin0,
        sched=sched0 + [dict(t, w=nwin0 + t["w"]) for t in sched1],
        ntiles0=ntiles0,
    )
    return in_maps, params


def _build_nc(prm):
    import concourse.bass as bass
    import concourse.mybir as mybir

    f_in, n_hid, n_cls = prm["f_in"], prm["n_hid"], prm["n_cls"]
    dpc0, dpc1 = prm["dpc0"], prm["dpc1"]
    nwin0 = prm["nwin0"]
    sched = prm["sched"]
    ntiles0 = prm["ntiles0"]
    ntiles = len(sched)
    nwin = nwin0 + 1

    ch0 = _chunks(f_in)      # e.g. [128,128,128,128,90]
    ch1 = _chunks(n_hid)     # [128, 128]
    NC0, NC1 = len(ch0), len(ch1)
    FPAD0, FPAD1 = NC0 * P, NC1 * P

    NBUF = 8

    # per-window bookkeeping (cumulative thresholds), shared by all cores
    w_tiles = [[] for _ in range(nwin)]
    for t, td in enumerate(sched):
        w_tiles[td["w"]].append(t)
    cum_tiles = np.cumsum([0] + [len(ts) for ts in w_tiles])
    ncopies_w = [2 * NC0 if w < nwin0 else 2 * NC1 for w in range(nwin)]
    cum_copies = np.cumsum([0] + ncopies_w)  # s_cp threshold after window w = cum_copies[w+1]


    nc = bass.Bass("TRN2", target_bir_lowering=False, debug=False,
                   num_devices=NCORES)

    x16_d = nc.dram_tensor("x16", [prm["n_src0"], f_in], mybir.dt.float16, kind="ExternalInput")
    srcidx_d = nc.dram_tensor("srcidx", [P, ntiles], mybir.dt.int32, kind="ExternalInput")
    dstv_d = nc.dram_tensor("dstv", [P, ntiles], mybir.dt.float32, kind="ExternalInput")
    valv_d = nc.dram_tensor("valv", [P, ntiles], mybir.dt.float32, kind="ExternalInput")
    W0s_d = nc.dram_tensor("W0s", [f_in + 1, n_hid], mybir.dt.float16, kind="ExternalInput")
    W0n_d = nc.dram_tensor("W0n", [f_in, n_hid], mybir.dt.float16, kind="ExternalInput")
    W1s_d = nc.dram_tensor("W1s", [n_hid + 1, n_cls], mybir.dt.float16, kind="ExternalInput")
    W1n_d = nc.dram_tensor("W1n", [n_hid, n_cls], mybir.dt.float16, kind="ExternalInput")
    out_d = nc.dram_tensor("out", [P, n_cls], mybir.dt.float32, kind="ExternalOutput")

    h_local = nc.dram_tensor("h_local", [dpc0, n_hid], mybir.dt.float16)
    h_full = nc.dram_tensor("h_full", [dpc0 * NCORES, n_hid], mybir.dt.float16)

    dt = mybir.dt
    AF = mybir.ActivationFunctionType
    AL = mybir.AluOpType

    from contextlib import ExitStack
    es = ExitStack()
    with es:
        block = es.enter_context(nc.Block())
        sem = lambda n: es.enter_context(nc.semaphore(n))
        sb = lambda n, shp, d: es.enter_context(nc.sbuf_tensor(n, shp, d))
        ps = lambda n, shp: es.enter_context(nc.psum_tensor(n, shp, dt.float32))
        s_init, s_iota, s_oh, s_pe, s_cp, s_wmm, s_hs, s_cc, s_od = (
            sem("s_init"), sem("s_iota"), sem("s_oh"), sem("s_pe"),
            sem("s_cp"), sem("s_wmm"), sem("s_hs"), sem("s_cc"), sem("s_od"))
        s_g = [sem(f"s_g{i}") for i in range(NBUF)]
        s_hd = [sem(f"s_hd{i}") for i in range(2)]
        G = sb("G", [P, NBUF * f_in], dt.float16)
        OH = sb("OH", [P, NBUF * P], dt.float16)
        srcidx = sb("srcidx_s", [P, ntiles], dt.int32)
        dstv = sb("dstv_s", [P, ntiles], dt.float32)
        valv = sb("valv_s", [P, ntiles], dt.float32)
        iota_i = sb("iota_i", [P, P], dt.int32)
        iota_f = sb("iota_f", [P, P], dt.float16)
        W0s_s = sb("W0s_s", [P, NC0 * n_hid], dt.float16)
        W0n_s = sb("W0n_s", [P, NC0 * n_hid], dt.float16)
        W1s_s = sb("W1s_s", [P, NC1 * n_cls], dt.float16)
        W1n_s = sb("W1n_s", [P, NC1 * n_cls], dt.float16)
        b1row = sb("b1row", [1, n_cls], dt.float16)
        ones1 = sb("ones1", [1, P], dt.float16)
        aggT = sb("aggT", [P, 2 * FPAD0], dt.float16)
        selfT = sb("selfT", [P, 2 * FPAD0], dt.float16)
        agg1T = sb("agg1T", [P, FPAD1], dt.float16)
        self1T = sb("self1T", [P, FPAD1], dt.float16)
        h_sb = sb("h_sb", [P, 2 * n_hid], dt.float16)
        out_sb = sb("out_sb", [P, n_cls], dt.float32)
        ps_agg = ps("ps_agg", [P, FPAD0])
        ps_self = ps("ps_self", [P, FPAD0])
        ps_h = ps("ps_h", [P, n_hid])
        ps_agg1 = ps("ps_agg1", [P, FPAD1])
        ps_self1 = ps("ps_self1", [P, FPAD1])
        ps_out = ps("ps_out", [P, n_cls])

        n_init = 0

        @block.gpsimd
        def _(g):
            nonlocal n_init
            # ---- initial loads ----
            def ld(dst_ap, src_ap):
                nonlocal n_init
                g.dma_start(out=dst_ap, in_=src_ap).then_inc(s_init, 16)
                n_init += 1
            ld(srcidx[:, :], srcidx_d[:, :])
            ld(dstv[:, :], dstv_d[:, :])
            ld(valv[:, :], valv_d[:, :])
            ofs = 0
            for c, kc in enumerate(ch0):
                ld(W0s_s[0:kc, c * n_hid:(c + 1) * n_hid], W0s_d[ofs:ofs + kc, :])
                ld(W0n_s[0:kc, c * n_hid:(c + 1) * n_hid], W0n_d[ofs:ofs + kc, :])
                ofs += kc
            # bias row of W0s goes to partition row kc of last chunk
            last = NC0 - 1
            ld(W0s_s[ch0[last]:ch0[last] + 1, last * n_hid:(last + 1) * n_hid],
               W0s_d[f_in:f_in + 1, :])
            ofs = 0
            for c, kc in enumerate(ch1):
                ld(W1s_s[0:kc, c * n_cls:(c + 1) * n_cls], W1s_d[ofs:ofs + kc, :])
                ld(W1n_s[0:kc, c * n_cls:(c + 1) * n_cls], W1n_d[ofs:ofs + kc, :])
                ofs += kc
            ld(b1row[0:1, :], W1s_d[n_hid:n_hid + 1, :])
            g.iota(iota_i[:, :], pattern=[[1, P]], base=0,
                   channel_multiplier=0).then_inc(s_iota, 1)
            g.wait_ge(s_init, 16 * n_init)

            # ---- gathers (L0 then L1), tile stream ----
            for t, td in enumerate(sched):
                if t == ntiles0:
                    # before L1 gathers: h must be stored fully
                    g.wait_ge(s_hd[0], 16 * ((nwin0 + 1) // 2))
                    g.wait_ge(s_hd[1], 16 * (nwin0 // 2))
                    g.collective_compute(
                        "AllGather",
                        AL.bypass,
                        replica_groups=[list(range(NCORES))],
                        ins=[h_local.ap().opt()],
                        outs=[h_full.ap().opt()],
                    ).then_inc(s_cc, 1)
                    g.wait_ge(s_cc, 1)
                if t >= NBUF:
                    g.wait_ge(s_pe, t + 1 - NBUF)
                b = t % NBUF
                if t < ntiles0:
                    g.indirect_dma_start(
                        out=G[:, b * f_in:(b + 1) * f_in], out_offset=None,
                        in_=x16_d[:, :],
                        in_offset=bass.IndirectOffsetOnAxis(ap=srcidx[:, t:t + 1], axis=0),
                    ).then_inc(s_g[t % NBUF], 16)
                else:
                    g.indirect_dma_start(
                        out=G[:, b * f_in:b * f_in + n_hid], out_offset=None,
                        in_=h_full[:, :],
                        in_offset=bass.IndirectOffsetOnAxis(ap=srcidx[:, t:t + 1], axis=0),
                    ).then_inc(s_g[t % NBUF], 16)

        @block.vector
        def _(v):
            v.wait_ge(s_init, 16 * n_init)
            v.wait_ge(s_iota, 1)
            v.tensor_copy(out=iota_f[:, :], in_=iota_i[:, :])
            v.memset(ones1[0:1, :], 1.0)
            # ones row for L0 self bias: partition ch0[-1]-... lives in selfT
            # chunk NC0-1 row ch0[-1] (i.e. the f_in-th K row) of BOTH buffers
            last = NC0 - 1
            krow = ch0[last]  # row index of ones within last chunk (e.g. 90)
            for bb in range(2):
                v.memset(selfT[:, bb * FPAD0 + last * P: bb * FPAD0 + (last + 1) * P], 1.0)
            v.drain()
            for t in range(ntiles):
                if t >= NBUF:
                    v.wait_ge(s_pe, t + 1 - NBUF)
                b = t % NBUF
                v.tensor_scalar(out=OH[:, b * P:(b + 1) * P], in0=iota_f[:, :],
                                scalar1=dstv[:, t:t + 1], scalar2=valv[:, t:t + 1],
                                op0=AL.is_equal, op1=AL.mult).then_inc(s_oh, 1)

        @block.tensor
        def _(t_):
            for w in range(nwin):
                is0 = w < nwin0
                nch = NC0 if is0 else NC1
                chs = ch0 if is0 else ch1
                fdim = f_in if is0 else n_hid
                pagg = ps_agg if is0 else ps_agg1
                pself = ps_self if is0 else ps_self1
                # psum WAW: previous window's copies must be done
                if w >= 1:
                    t_.wait_ge(s_cp, int(cum_copies[w]))
                # bank layout of chunk outputs: chunk c -> bank (c*P*4)//2048
                banks = [(c * P * 4) // 2048 for c in range(nch)]
                first_c = {b: min(c for c in range(nch) if banks[c] == b) for b in set(banks)}
                last_c = {b: max(c for c in range(nch) if banks[c] == b) for b in set(banks)}
                tiles = w_tiles[w]
                n_agg = len(tiles) - 1
                for j, t in enumerate(tiles):
                    td = sched[t]
                    t_.wait_ge(s_g[t % NBUF], 16 * (t // NBUF + 1))
                    t_.wait_ge(s_oh, t + 1)
                    b = t % NBUF
                    is_self = td["kind"] == "self"
                    tgt = pself if is_self else pagg
                    first = True if is_self else (j == 0)
                    lastt = True if is_self else (j == n_agg - 1)
                    fofs = 0
                    for c in range(nch):
                        mc = chs[c]
                        mm = t_.matmul(
                            out=tgt[0:mc, c * P:c * P + P],
                            lhsT=G[:, b * f_in + fofs: b * f_in + fofs + mc],
                            rhs=OH[:, b * P:(b + 1) * P],
                            start=first and (c == first_c[banks[c]]),
                            stop=lastt and (c == last_c[banks[c]]))
                        fofs += mc
                    mm.then_inc(s_pe, 1)
                # W matmuls after ACT copied this window's psums to SBUF
                t_.wait_ge(s_cp, int(cum_copies[w + 1]))
                t_.wait_ge(s_hs, w)  # ACT done with previous window's ps_h/ps_out
                bb = (w % 2) if is0 else 0
                a_sb = aggT if is0 else agg1T
                s_sb = selfT if is0 else self1T
                a_ofs = bb * FPAD0 if is0 else 0
                Ws = W0s_s if is0 else W1s_s
                Wn = W0n_s if is0 else W1n_s
                ncol = n_hid if is0 else n_cls
                pout = ps_h if is0 else ps_out
                mdst = P if is0 else dpc1
                nmm = 2 * nch + (0 if is0 else 1)
                k = 0
                for c in range(nch):
                    kc = chs[c] + (1 if (is0 and c == nch - 1) else 0)
                    mm = t_.matmul(out=pout[0:mdst, 0:ncol],
                                   lhsT=s_sb[0:kc, a_ofs + c * P: a_ofs + c * P + mdst],
                                   rhs=Ws[0:kc, c * ncol:(c + 1) * ncol],
                                   start=(k == 0), stop=False)
                    k += 1
                if not is0:
                    mm = t_.matmul(out=pout[0:mdst, 0:ncol],
                                   lhsT=ones1[0:1, 0:mdst],
                                   rhs=b1row[0:1, 0:ncol],
                                   start=False, stop=False)
                    k += 1
                for c in range(nch):
                    kc = chs[c]
                    mm = t_.matmul(out=pout[0:mdst, 0:ncol],
                                   lhsT=a_sb[0:kc, a_ofs + c * P: a_ofs + c * P + mdst],
                                   rhs=Wn[0:kc, c * ncol:(c + 1) * ncol],
                                   start=False, stop=(k == nmm - 1))
                    k += 1
                mm.then_inc(s_wmm, 1)

        @block.scalar
        def _(s):
            for w in range(nwin):
                is0 = w < nwin0
                nch = NC0 if is0 else NC1
                chs = ch0 if is0 else ch1
                pagg = ps_agg if is0 else ps_agg1
                pself = ps_self if is0 else ps_self1
                a_sb = aggT if is0 else agg1T
                s_sb = selfT if is0 else self1T
                bb = (w % 2) if is0 else 0
                a_ofs = bb * FPAD0 if is0 else 0
                s.wait_ge(s_pe, int(cum_tiles[w + 1]))
                if is0 and w >= 2:
                    s.wait_ge(s_wmm, w - 1)  # PE done reading buffer bb
                if not is0:
                    s.wait_ge(s_wmm, w)
                for c in range(nch):
                    mc = chs[c]
                    s.activation(out=a_sb[0:mc, a_ofs + c * P: a_ofs + c * P + P],
                                 in_=pagg[0:mc, c * P:c * P + P], func=AF.Copy).then_inc(s_cp, 1)
                for c in range(nch):
                    mc = chs[c]  # for L0 last chunk: copy only mc rows, ones row preserved
                    s.activation(out=s_sb[0:mc, a_ofs + c * P: a_ofs + c * P + P],
                                 in_=pself[0:mc, c * P:c * P + P], func=AF.Copy).then_inc(s_cp, 1)
                s.wait_ge(s_wmm, w + 1)
                if is0:
                    if w >= 2:
                        s.wait_ge(s_hd[w % 2], 16 * ((w - 2) // 2 + 1))  # h_sb reuse
                    s.activation(out=h_sb[:, (w % 2) * n_hid:(w % 2 + 1) * n_hid],
                                 in_=ps_h[:, :], func=AF.Relu).then_inc(s_hs, 1)
                else:
                    s.activation(out=out_sb[0:dpc1, :], in_=ps_out[0:dpc1, :],
                                 func=AF.Copy).then_inc(s_hs, 1)

        @block.sync
        def _(sp):
            for w in range(nwin0):
                sp.wait_ge(s_hs, w + 1)
                rows = min(P, dpc0 - w * P)
                sp.dma_start(out=h_local[w * P: w * P + rows, :],
                             in_=h_sb[0:rows, (w % 2) * n_hid:(w % 2) * n_hid + n_hid]
                             ).then_inc(s_hd[w % 2], 16)
            sp.wait_ge(s_hs, nwin)
            sp.dma_start(out=out_d[0:dpc1, :], in_=out_sb[0:dpc1, :]).then_inc(s_od, 16)
            sp.wait_ge(s_od, 16)

    return nc


def _run(inputs, dims, trace=False):
    from concourse.bass_utils import run_bass_kernel_spmd
    in_maps, params = _preprocess(**inputs, **dims)
    nc = _build_nc(dict(params, **{k: dims[k] for k in
                                   ("n_src0", "f_in", "n_hid", "n_cls")}))
    res = run_bass_kernel_spmd(nc, in_maps, core_ids=list(range(NCORES)),
                               trace=trace)
    dpc1 = dims["n_dst1"] // NCORES
    out = np.concatenate([res.results[c]["out"][:dpc1] for c in range(NCORES)], 0)
    return out.astype(np.float32), res


def kernel(**inputs):
    dims = dict(n_src0=N_SRC0, n_dst0=N_DST0, n_dst1=N_DST1,
                f_in=F_IN, n_hid=N_HID, n_cls=N_CLS)
    out, _ = _run(inputs, dims)
    return out

